# revision 1
# baseline (speedup 1.0000x reference)
"""DeepseekV2 MLA attention for 8 TRN2 NeuronCores (Bass/Tile).

Sharding: core c handles batch b=c//4, head-group g=c%4 (4 of 16 heads).
The q_a/kv_a projections + shared kv latent / k_pe are replicated within
each batch's 4 cores (MLA's point); o_proj is row-parallel with the
4 partial outputs summed on the host during the gather step.

Dataflow is fully "transposed-chain": activations live as [feature, token]
tiles so attention scores are computed directly in [k_tok, q_tok] layout
(no on-chip transposes), softmax runs without max-subtraction (logits are
O(1) by construction), and all per-token normalizations (rmsnorm scale,
softmax 1/sum) are scale rows broadcast across partitions. rotate_half is
a constant 128x128 matmul. o_proj is computed transposed ([o_dim, token]).

All GEMM operands are bf16 (PSUM accumulation stays fp32) and every weight
is pre-packed host-side into the exact SBUF tile layout so each load is a
single fully-contiguous DMA, issued once and kept resident for the whole
kernel.
"""

import numpy as np

import concourse.bacc as bacc
import concourse.mybir as mybir
import concourse.tile as tile
from concourse.bass_utils import run_bass_kernel_spmd

F32 = mybir.dt.float32
BF16 = mybir.dt.bfloat16

# problem constants
B, S, HID, QL = 2, 1024, 2048, 1536
NH, NOPE, ROPE, VD, KVL = 16, 128, 64, 128, 512
QHD = NOPE + ROPE  # 192
EPS = 1e-6
THETA = 10000.0
HG = 4          # heads per core
GW = HG * VD    # 512, attn-cat width per core
NKT = HID // 128   # 16 contraction tiles over hidden
NQL = QL // 128    # 12
HJ = S // 2        # 512 token half

MM_NP = np.float32


import ml_dtypes

BF16_NP = ml_dtypes.bfloat16


def _to_bf16_bits(a):
    """fp32 -> bf16 numpy (round-to-nearest-even)."""
    return np.ascontiguousarray(a, np.float32).astype(BF16_NP)


def _interleave_rows(w):
    # fold _interleave_perm into weight rows: out feature j = in feature perm[j]
    return np.concatenate([w[0::2], w[1::2]], axis=0)


def _rope_tables(positions):
    inv = 1.0 / (THETA ** (np.arange(0, ROPE, 2, dtype=np.float32) / ROPE))
    t = positions.astype(np.float32)
    freqs = np.outer(t, inv)
    emb = np.concatenate([freqs, freqs], axis=-1)  # [S, 64]
    return np.cos(emb), np.sin(emb)


def _rot_matrix():
    # R @ x = rotate_half(x) for 64-dim x; block-diag twice for 128 rows.
    R = np.zeros((ROPE, ROPE), np.float32)
    for j in range(32):
        R[j, j + 32] = -1.0
        R[j + 32, j] = 1.0
    R2 = np.zeros((128, 128), np.float32)
    R2[:64, :64] = R
    R2[64:, 64:] = R
    return R2


def _pack_stripes(wT, nk, nm):
    """wT [nk*128, nm*128] -> [128, nm, nk, 128] partition-major stripe pack."""
    K, M = wT.shape
    assert K == nk * 128 and M == nm * 128
    a = wT.reshape(nk, 128, nm, 128)          # k, p, m, f
    return np.ascontiguousarray(a.transpose(1, 2, 0, 3))  # p, m, k, f


def prep_in_maps(inputs):
    """Full inputs -> list of 8 per-core input dicts (numpy, host-side)."""
    h = np.asarray(inputs["hidden_states"], np.float32)
    pos = np.asarray(inputs["position_ids"])
    q_a_w = np.asarray(inputs["q_a_w"], np.float32)
    q_a_ln = np.asarray(inputs["q_a_ln"], np.float32)
    q_b_w = np.asarray(inputs["q_b_w"], np.float32)
    kv_a_w = np.asarray(inputs["kv_a_w"], np.float32)
    kv_a_ln = np.asarray(inputs["kv_a_ln"], np.float32)
    kv_b_w = np.asarray(inputs["kv_b_w"], np.float32)
    o_w = np.asarray(inputs["o_w"], np.float32)

    # q_a: [128, 12, 16, 128] stripe pack of q_a_w.T
    wqaP = _to_bf16_bits(_pack_stripes(q_a_w.T, NKT, NQL).reshape(128, NQL * NKT * 128))

    # kv_a: fold interleave perm into the k_pe rows (last 64), duplicate the
    # pe block so k_pe^T materializes on both partition halves. 5 stripes.
    pe_rows_w = _interleave_rows(kv_a_w[KVL:])
    kv_a_w2 = np.concatenate([kv_a_w[:KVL], pe_rows_w, pe_rows_w], axis=0)  # [640, HID]
    wkvaP = _to_bf16_bits(_pack_stripes(kv_a_w2.T, NKT, 5).reshape(128, 5 * NKT * 128))

    scale = QHD ** -0.5
    rotP = _to_bf16_bits(_rot_matrix().T)

    per_core = []
    for c in range(8):
        b, g = divmod(c, 4)
        heads = range(HG * g, HG * g + HG)

        # q_b rows for this group, blocked [4x nope(128), 2x pe-pair(128)],
        # with q_a_ln folded into columns, interleave perm folded into pe
        # rows, and the attention scale folded in.
        nope_rows = []
        pe_rows = []
        for hh in heads:
            rows = q_b_w[hh * QHD:(hh + 1) * QHD]  # [192, QL]
            nope_rows.append(rows[:NOPE])
            pe_rows.append(_interleave_rows(rows[NOPE:]))
        wqb_g = np.concatenate(nope_rows + pe_rows, axis=0)  # [768, QL]
        wqb_g = wqb_g * q_a_ln[None, :] * scale
        wqbP = _to_bf16_bits(_pack_stripes(wqb_g.T, NQL, 6).reshape(128, 6 * NQL * 128))

        # kv_b nope/v for this group with kv_a_ln folded; pack [128, 4, 512]
        kn_rows = []
        v_rows = []
        for hh in heads:
            rows = kv_b_w[hh * (NOPE + VD):(hh + 1) * (NOPE + VD)]
            kn_rows.append(rows[:NOPE])
            v_rows.append(rows[NOPE:])
        wkbn = (np.concatenate(kn_rows, axis=0) * kv_a_ln[None, :]).T  # [KVL, 512]
        wkbv = (np.concatenate(v_rows, axis=0) * kv_a_ln[None, :]).T
        kbnP = _to_bf16_bits(
            np.ascontiguousarray(wkbn.reshape(4, 128, GW).transpose(1, 0, 2)).reshape(128, 4 * GW))
        kbvP = _to_bf16_bits(
            np.ascontiguousarray(wkbv.reshape(4, 128, GW).transpose(1, 0, 2)).reshape(128, 4 * GW))

        # o_w columns for this group's heads: [128, 16, 4, 128] pack of wo^T
        wo = o_w[:, GW * g: GW * (g + 1)].T  # [512, HID]
        woP = _to_bf16_bits(_pack_stripes(wo, HG, NKT).reshape(128, NKT * HG * 128))

        # hidden transposed, packed per-j: [128, 2, 16, 512]
        hT = h[b].T                                   # [HID, S]
        hTP = hT.reshape(NKT, 128, 2, HJ).transpose(1, 2, 0, 3)  # p, j, k, t
        hTP = _to_bf16_bits(np.ascontiguousarray(hTP).reshape(128, 2 * NKT * HJ))

        cos, sin = _rope_tables(np.asarray(pos[b]))
        cosP = _to_bf16_bits(np.concatenate([cos.T, cos.T], axis=0))  # [128, S]
        sinP = _to_bf16_bits(np.concatenate([sin.T, sin.T], axis=0))

        per_core.append({
            "hTP": hTP, "wqaP": wqaP, "wqbP": wqbP, "wkvaP": wkvaP,
            "kbnP": kbnP, "kbvP": kbvP, "woP": woP,
            "cosP": cosP, "sinP": sinP, "rotP": rotP,
        })
    return per_core


def combine_outputs(results):
    """8 per-core outP [128, 2, 16, 512] bf16-bit partials -> [B, S, HID]."""
    out = np.zeros((B, S, HID), np.float32)
    for c, r in enumerate(results):
        b = c // 4
        f32 = np.asarray(r["outP"]).astype(np.float32).reshape(128, 2, NKT, HJ)
        # out[b, 512j+t, 128oc+p] += f32[p, j, oc, t]
        out[b] += f32.transpose(1, 3, 2, 0).reshape(S, HID)
    return out


def build_nc(debug=False, reps=1):
    nc = bacc.Bacc("TRN2", target_bir_lowering=False, debug=False, num_devices=8)
    dram = nc.declare_dram_parameter

    hTP = dram("hTP", [128, 2 * NKT * HJ], BF16, isOutput=False)
    wqaP = dram("wqaP", [128, NQL * NKT * 128], BF16, isOutput=False)
    wqbP = dram("wqbP", [128, 6 * NQL * 128], BF16, isOutput=False)
    wkvaP = dram("wkvaP", [128, 5 * NKT * 128], BF16, isOutput=False)
    kbnP = dram("kbnP", [128, 4 * GW], BF16, isOutput=False)
    kbvP = dram("kbvP", [128, 4 * GW], BF16, isOutput=False)
    woP = dram("woP", [128, NKT * HG * 128], BF16, isOutput=False)
    cosP = dram("cosP", [128, S], BF16, isOutput=False)
    sinP = dram("sinP", [128, S], BF16, isOutput=False)
    rotP = dram("rotP", [128, 128], BF16, isOutput=False)
    outP = dram("outP", [128, 2 * NKT * HJ], BF16, isOutput=True)

    AF = mybir.ActivationFunctionType
    MULT = mybir.AluOpType.mult
    ADD = mybir.AluOpType.add

    with tile.TileContext(nc) as tc:
        with (
            tc.tile_pool(name="consts", bufs=1) as consts,
            tc.tile_pool(name="sb", bufs=1) as sb,
            tc.tile_pool(name="ps", space="PSUM", bufs=1) as ps,
        ):
            # ---- weights, resident for the whole kernel. DMA issue order =
            # arrival order: S1(j0) needs ht0 + the first wqa stripes first,
            # so those go ahead of the bulk weights.
            # interleaved fine-grained prefetch: the first S1 chain needs
            # ht0's leading k-tiles + wqa stripe 0 only, so those land first.
            ht0 = sb.tile([128, NKT * HJ], BF16, name="ht0", tag="ht", bufs=1)
            wqa_sb = consts.tile([128, NQL * NKT * 128], BF16, name="wqa_sb")
            HC = NKT * HJ // 4          # 4 k-tiles of hidden
            SW = NKT * 128              # one wqa stripe
            nc.sync.dma_start(ht0[:, 0:HC], hTP[:, 0:HC])
            nc.sync.dma_start(wqa_sb[:, 0:SW], wqaP[:, 0:SW])
            nc.sync.dma_start(ht0[:, HC:2 * HC], hTP[:, HC:2 * HC])
            nc.sync.dma_start(wqa_sb[:, SW:4 * SW], wqaP[:, SW:4 * SW])
            nc.sync.dma_start(ht0[:, 2 * HC:3 * HC], hTP[:, 2 * HC:3 * HC])
            nc.sync.dma_start(ht0[:, 3 * HC:4 * HC], hTP[:, 3 * HC:4 * HC])
            nc.sync.dma_start(wqa_sb[:, 4 * SW:8 * SW], wqaP[:, 4 * SW:8 * SW])
            nc.sync.dma_start(wqa_sb[:, 8 * SW:], wqaP[:, 8 * SW:])
            wkva_sb = consts.tile([128, 5 * NKT * 128], BF16, name="wkva_sb")
            nc.sync.dma_start(wkva_sb[:], wkvaP[:, :])
            kbn_sb = consts.tile([128, 4 * GW], BF16, name="kbn_sb")
            nc.sync.dma_start(kbn_sb[:], kbnP[:, :])
            kbv_sb = consts.tile([128, 4 * GW], BF16, name="kbv_sb")
            nc.sync.dma_start(kbv_sb[:], kbvP[:, :])
            wqb_sb = consts.tile([128, 6 * NQL * 128], BF16, name="wqb_sb")
            nc.sync.dma_start(wqb_sb[:], wqbP[:, :])
            wo_sb = consts.tile([128, NKT * HG * 128], BF16, name="wo_sb")
            nc.sync.dma_start(wo_sb[:], woP[:, :])

            # ---- constants ----
            ones_f = consts.tile([128, 1], F32, name="ones_f")
            nc.vector.memset(ones_f[:], 1.0)
            ones_b = consts.tile([128, 1], BF16, name="ones_b")
            nc.vector.tensor_copy(ones_b[:], ones_f[:])
            eps_sb = consts.tile([128, 1], F32, name="eps_sb")
            nc.vector.memset(eps_sb[:], EPS)
            rot_sb = consts.tile([128, 128], BF16, name="rot_sb")
            nc.sync.dma_start(rot_sb[:], rotP[:, :])
            cos_sb = consts.tile([128, S], BF16, name="cos_sb")
            sin_sb = consts.tile([128, S], BF16, name="sin_sb")
            nc.sync.dma_start(cos_sb[:], cosP[:, :])
            nc.sync.dma_start(sin_sb[:], sinP[:, :])
            # causal mask tiles: cmask[t][p, x] = 1.0 if x - p >= 128*t else 0
            cmask = []
            for t in range(4):
                mt = consts.tile([128, HJ], BF16, name=f"cmask{t}")
                nc.gpsimd.memset(mt[:], 1.0)
                nc.gpsimd.affine_select(
                    out=mt[:], in_=mt[:],
                    compare_op=mybir.AluOpType.is_ge, fill=0.0,
                    base=-128 * t, pattern=[[1, HJ]], channel_multiplier=-1)
                cmask.append(mt)

            # expT ring slots are partially written on diagonal tiles; the
            # masked left region reads stale slot content times zero, so the
            # slots must start finite.
            et_init = []
            for sl in range(4):
                ei = sb.tile([128, HJ], BF16, name=f"et_init{sl}", tag="expT", bufs=4)
                nc.vector.memset(ei[:], 0.0)

            # ---- persistent k-side tensors (full S) ----
            kpeT = sb.tile([128, S], BF16, name="kpeT", tag="kpeT", bufs=1)
            kT = [sb.tile([128, S], BF16, name=f"kT{hh}", tag="kT", bufs=4)
                  for hh in range(HG)]
            vsb = [sb.tile([128, GW], BF16, name=f"v{i}", tag="v", bufs=8)
                   for i in range(8)]

            for rep in range(reps):
                for j in range(2):
                    jsl = slice(j * HJ, (j + 1) * HJ)

                    # ---- hidden half (j0 of rep0 was prefetched first) ----
                    if j == 0 and rep == 0:
                        ht = ht0
                    else:
                        ht = sb.tile([128, NKT * HJ], BF16, name=f"htr{rep}_{j}", tag="ht", bufs=1)
                        nc.sync.dma_start(ht[:], hTP[:, j * NKT * HJ:(j + 1) * NKT * HJ])

                    def htk(k):
                        return ht[:, k * HJ:(k + 1) * HJ]

                    # ---- S1: q_lat^T = q_a_w @ h^T (12 stripes) + sum of squares ----
                    ql_t = []
                    ps_msq = ps.tile([1, HJ], F32, name=f"msq_q{j}", tag="row", bufs=2)
                    for m in range(NQL):
                        pm = ps.tile([128, HJ], F32, name=f"ps_qa{j}_{m}", tag="mm", bufs=4)
                        for k in range(NKT):
                            nc.tensor.matmul(pm[:], wqa_sb[:, (m * NKT + k) * 128:(m * NKT + k + 1) * 128],
                                             htk(k), start=(k == 0), stop=(k == NKT - 1))
                        qt = sb.tile([128, HJ], BF16, name=f"ql{j}_{m}", tag="ql", bufs=NQL)
                        nc.vector.tensor_copy(qt[:], pm[:])
                        sqt = sb.tile([128, HJ], BF16, name=f"sq_q{j}_{m}", tag="sqt", bufs=2)
                        nc.scalar.activation(sqt[:], pm[:], AF.Square)
                        nc.tensor.matmul(ps_msq[:], ones_b[:], sqt[:],
                                         start=(m == 0), stop=(m == NQL - 1))
                        ql_t.append(qt)

                    # rmsnorm scale row for q (applied at the q^T stage)
                    sr_q = sb.tile([1, HJ], F32, name=f"sr_q{j}", tag="srow", bufs=2)
                    nc.scalar.activation(sr_q[:], ps_msq[:], AF.Sqrt, bias=eps_sb[0:1, :], scale=1.0 / QL)
                    rr_q = sb.tile([1, HJ], F32, name=f"rr_q{j}", tag="srow", bufs=2)
                    nc.vector.reciprocal(rr_q[:], sr_q[:])
                    R_q = sb.tile([128, HJ], F32, name=f"R_q{j}", tag="bcast", bufs=2)
                    nc.gpsimd.partition_broadcast(R_q[:], rr_q[:])

                    # ---- S3: kv_a -> latent (4 stripes) + k_pe (64 rows) ----
                    latn = []
                    ps_msk = ps.tile([1, HJ], F32, name=f"msq_kv{j}", tag="row", bufs=2)
                    for m in range(4):
                        pm = ps.tile([128, HJ], F32, name=f"ps_kva{j}_{m}", tag="mm", bufs=4)
                        for k in range(NKT):
                            nc.tensor.matmul(pm[:], wkva_sb[:, (m * NKT + k) * 128:(m * NKT + k + 1) * 128],
                                             htk(k), start=(k == 0), stop=(k == NKT - 1))
                        lt = sb.tile([128, HJ], BF16, name=f"latn{j}_{m}", tag="latn", bufs=4)
                        nc.vector.tensor_copy(lt[:], pm[:])
                        latn.append(lt)
                        sqt = sb.tile([128, HJ], BF16, name=f"sq_kv{j}_{m}", tag="sqt", bufs=2)
                        nc.scalar.activation(sqt[:], pm[:], AF.Square)
                        nc.tensor.matmul(ps_msk[:], ones_b[:], sqt[:],
                                         start=(m == 0), stop=(m == 3))
                    sr_k = sb.tile([1, HJ], F32, name=f"sr_k{j}", tag="srow", bufs=2)
                    nc.scalar.activation(sr_k[:], ps_msk[:], AF.Sqrt, bias=eps_sb[0:1, :], scale=1.0 / KVL)
                    rr_k = sb.tile([1, HJ], F32, name=f"rr_k{j}", tag="srow", bufs=2)
                    nc.vector.reciprocal(rr_k[:], sr_k[:])
                    R_kv = sb.tile([128, HJ], F32, name=f"R_kv{j}", tag="bcast", bufs=2)
                    nc.gpsimd.partition_broadcast(R_kv[:], rr_k[:])
                    latb = latn
                    for m in range(4):
                        nc.vector.tensor_tensor(out=latn[m][:], in0=latn[m][:], in1=R_kv[:], op=MULT)

                    # k_pe stripe: matmul (dup'd 2x64 rows) + rope, no norm
                    pm = ps.tile([128, HJ], F32, name=f"ps_pe{j}", tag="mm", bufs=4)
                    for k in range(NKT):
                        nc.tensor.matmul(pm[:], wkva_sb[:, (4 * NKT + k) * 128:(4 * NKT + k + 1) * 128],
                                         htk(k), start=(k == 0), stop=(k == NKT - 1))
                    xpe = sb.tile([128, HJ], BF16, name=f"xpe{j}", tag="tmp", bufs=4)
                    nc.vector.tensor_copy(xpe[:], pm[:])
                    pr = ps.tile([128, HJ], F32, name=f"ps_rot{j}", tag="mm", bufs=4)
                    nc.tensor.matmul(pr[:], rot_sb[:], xpe[:], start=True, stop=True)
                    t1 = sb.tile([128, HJ], BF16, name=f"t1k{j}", tag="tmp", bufs=4)
                    nc.vector.tensor_tensor(out=t1[:], in0=xpe[:], in1=cos_sb[:, jsl], op=MULT)
                    t2 = sb.tile([128, HJ], BF16, name=f"t2k{j}", tag="tmp", bufs=4)
                    nc.vector.tensor_tensor(out=t2[:], in0=pr[:], in1=sin_sb[:, jsl], op=MULT)
                    nc.vector.tensor_tensor(out=kpeT[:, jsl], in0=t1[:], in1=t2[:], op=ADD)

                    # ---- S4: k_nope^T per head ----
                    for hh in range(HG):
                        pm = ps.tile([128, HJ], F32, name=f"ps_kn{j}_{hh}", tag="mm", bufs=4)
                        for k4 in range(4):
                            nc.tensor.matmul(pm[:], kbn_sb[:, k4 * GW + hh * 128:k4 * GW + (hh + 1) * 128],
                                             latb[k4][:], start=(k4 == 0), stop=(k4 == 3))
                        nc.scalar.copy(kT[hh][:, jsl], pm[:])

                    # ---- S5: v (natural layout) per 128-token chunk ----
                    for tt in range(4):
                        i = 4 * j + tt
                        csl = slice(tt * 128, (tt + 1) * 128)
                        pm = ps.tile([128, GW], F32, name=f"ps_v{i}", tag="mm", bufs=4)
                        for k4 in range(4):
                            nc.tensor.matmul(pm[:], latb[k4][:, csl], kbv_sb[:, k4 * GW:(k4 + 1) * GW],
                                             start=(k4 == 0), stop=(k4 == 3))
                        nc.scalar.copy(vsb[i][:], pm[:])

                    # ---- S2: q^T stripes (4 nope heads + 2 pe pairs) ----
                    qT = []
                    for m in range(6):
                        pm = ps.tile([128, HJ], F32, name=f"ps_qb{j}_{m}", tag="mm", bufs=4)
                        for k in range(NQL):
                            nc.tensor.matmul(pm[:], wqb_sb[:, (m * NQL + k) * 128:(m * NQL + k + 1) * 128],
                                             ql_t[k][:], start=(k == 0), stop=(k == NQL - 1))
                        qt = sb.tile([128, HJ], BF16, name=f"qT{j}_{m}", tag="qT", bufs=6)
                        if m < 4:
                            nc.vector.tensor_tensor(out=qt[:], in0=pm[:], in1=R_q[:], op=MULT)
                        else:
                            xq = sb.tile([128, HJ], BF16, name=f"xq{j}_{m}", tag="tmp", bufs=4)
                            nc.vector.tensor_copy(xq[:], pm[:])
                            pr = ps.tile([128, HJ], F32, name=f"ps_rotq{j}_{m}", tag="mm", bufs=4)
                            nc.tensor.matmul(pr[:], rot_sb[:], xq[:], start=True, stop=True)
                            t1 = sb.tile([128, HJ], BF16, name=f"t1q{j}_{m}", tag="tmp", bufs=4)
                            nc.vector.tensor_tensor(out=t1[:], in0=xq[:], in1=cos_sb[:, jsl], op=MULT)
                            t2 = sb.tile([128, HJ], BF16, name=f"t2q{j}_{m}", tag="tmp", bufs=4)
                            nc.vector.tensor_tensor(out=t2[:], in0=pr[:], in1=sin_sb[:, jsl], op=MULT)
                            t3 = sb.tile([128, HJ], BF16, name=f"t3q{j}_{m}", tag="tmp", bufs=4)
                            nc.vector.tensor_tensor(out=t3[:], in0=t1[:], in1=t2[:], op=ADD)
                            nc.vector.tensor_tensor(out=qt[:], in0=t3[:], in1=R_q[:], op=MULT)
                        qT.append(qt)

                    # ---- S6/S7: attention per head ----
                    attn = []
                    for hh in range(HG):
                        qpe = qT[4 + hh // 2][(hh % 2) * 64:(hh % 2) * 64 + 64, :]
                        po = ps.tile([128, HJ], F32, name=f"ps_o{j}_{hh}", tag="acc", bufs=2)
                        psum = ps.tile([1, HJ], F32, name=f"ps_sum{j}_{hh}", tag="row", bufs=2)
                        irange = list(range(4 * (j + 1)))
                        last = irange[-1]
                        for i in irange:
                            t = i - 4 * j
                            q0 = t * 128 if t > 0 else 0  # columns < t*128 are fully masked
                            qs = slice(q0, HJ)
                            pss = ps.tile([128, HJ], F32, name=f"ps_s{j}_{hh}_{i}", tag="mm", bufs=4)
                            nc.tensor.matmul(pss[:, qs], kT[hh][:, i * 128:(i + 1) * 128], qT[hh][:, qs],
                                             start=True, stop=False)
                            pe0 = (hh % 2) * 64
                            nc.tensor.matmul(pss[:, qs], kpeT[pe0:pe0 + 64, i * 128:(i + 1) * 128],
                                             qpe[:, qs], start=False, stop=True)
                            et = sb.tile([128, HJ], BF16, name=f"e{j}_{hh}_{i}", tag="expT", bufs=4)
                            nc.scalar.activation(et[:, qs], pss[:, qs], AF.Exp)
                            if i * 128 + 127 > j * HJ:  # diagonal-crossing tile: causal mask
                                # full-range multiply also zeroes the unwritten
                                # left columns (mask is 0 there)
                                nc.vector.tensor_tensor(out=et[:], in0=et[:], in1=cmask[i - 4 * j][:], op=MULT)
                            nc.tensor.matmul(psum[:], ones_b[:], et[:],
                                             start=(i == 0), stop=(i == last))
                            nc.tensor.matmul(po[:], vsb[i][:, hh * 128:(hh + 1) * 128], et[:],
                                             start=(i == 0), stop=(i == last))
                        rs = sb.tile([1, HJ], F32, name=f"rs{j}_{hh}", tag="srow", bufs=2)
                        nc.vector.reciprocal(rs[:], psum[:])
                        Rs = sb.tile([128, HJ], F32, name=f"Rs{j}_{hh}", tag="bcast", bufs=2)
                        nc.gpsimd.partition_broadcast(Rs[:], rs[:])
                        at = sb.tile([128, HJ], BF16, name=f"attn{j}_{hh}", tag="attn", bufs=4)
                        nc.vector.tensor_tensor(out=at[:], in0=po[:], in1=Rs[:], op=MULT)
                        attn.append(at)

                    # ---- S8: o_proj transposed: outP[o, t] = sum_c wo[c, o] attn_catT[c, t] ----
                    for oc4 in range(4):
                        ot = sb.tile([128, 4 * HJ], BF16, name=f"ot{j}_{oc4}", tag="osb", bufs=2)
                        for q4 in range(4):
                            oc = oc4 * 4 + q4
                            pm = ps.tile([128, HJ], F32, name=f"ps_out{j}_{oc}", tag="mm", bufs=4)
                            for hh in range(HG):
                                nc.tensor.matmul(pm[:], wo_sb[:, oc * GW + hh * 128:oc * GW + (hh + 1) * 128],
                                                 attn[hh][:], start=(hh == 0), stop=(hh == HG - 1))
                            nc.vector.tensor_copy(ot[:, q4 * HJ:(q4 + 1) * HJ], pm[:])
                        nc.sync.dma_start(
                            outP[:, (j * NKT + oc4 * 4) * HJ:(j * NKT + oc4 * 4 + 4) * HJ],
                            ot[:])

    nc.compile()
    return nc


_NC = None


def _get_nc():
    global _NC
    if _NC is None:
        _NC = build_nc()
    return _NC


def run(inputs, trace=False):
    in_maps = prep_in_maps(inputs)
    nc = _get_nc()
    res = run_bass_kernel_spmd(nc, in_maps, core_ids=list(range(8)), trace=trace)
    out = combine_outputs(res.results)
    return out, res


def kernel(**inputs):
    out, _ = run(inputs)
    return out.astype(np.float32)



# revision 2
# speedup vs baseline: 1.4982x; 1.4982x over previous
"""DeepseekV2 MLA attention, v2: token-sharded q_a/kv_a + per-group AllGather.

Core c: batch b=c//4, head-group g=c%4, token-quarter g (256 tokens).
Per rep each core runs q_a + rmsnorm (normalization folded into ql) and
kv_a + rmsnorm + k_pe rope for ITS quarter only, then a 4-rank AllGather
exchanges the 17 normalized stripes (12 ql + 4 latent + 1 k_pe).  q_b,
kv_b, attention and o_proj then run exactly as v1 on the full sequence for
the core's own 4 heads (o_proj row-parallel, host-side group sum).

This removes the 4x replication of q_a/kv_a (51% of v1's PE columns) at the
cost of one ~1.1MB->4.5MB AllGather per rep, which overlaps the previous
rep's attention/o_proj.  wqa streams per-rep in quarters to fit SBUF.
"""

import numpy as np

import concourse.bacc as bacc
import concourse.mybir as mybir
import concourse.tile as tile
from concourse.bass_utils import run_bass_kernel_spmd

F32 = mybir.dt.float32
BF16 = mybir.dt.bfloat16

B, S, HID, QL = 2, 1024, 2048, 1536
NH, NOPE, ROPE, VD, KVL = 16, 128, 64, 128, 512
QHD = NOPE + ROPE
EPS = 1e-6
THETA = 10000.0
HG = 4
GW = HG * VD          # 512
NKT = HID // 128      # 16
NQL = QL // 128       # 12
HJ = S // 2           # 512
QT = S // 4           # 256-token quarter
NST = 17              # gathered stripes: 12 ql + 4 latn + 1 kpe

import ml_dtypes

BF16_NP = ml_dtypes.bfloat16


def _to_bf16_bits(a):
    return np.ascontiguousarray(a, np.float32).astype(BF16_NP)


def _interleave_rows(w):
    return np.concatenate([w[0::2], w[1::2]], axis=0)


def _rope_tables(positions):
    inv = 1.0 / (THETA ** (np.arange(0, ROPE, 2, dtype=np.float32) / ROPE))
    t = positions.astype(np.float32)
    freqs = np.outer(t, inv)
    emb = np.concatenate([freqs, freqs], axis=-1)
    return np.cos(emb), np.sin(emb)


def _rot_matrix():
    R = np.zeros((ROPE, ROPE), np.float32)
    for j in range(32):
        R[j, j + 32] = -1.0
        R[j + 32, j] = 1.0
    R2 = np.zeros((128, 128), np.float32)
    R2[:64, :64] = R
    R2[64:, 64:] = R
    return R2


def _pack_stripes(wT, nk, nm):
    K, M = wT.shape
    assert K == nk * 128 and M == nm * 128
    a = wT.reshape(nk, 128, nm, 128)
    return np.ascontiguousarray(a.transpose(1, 2, 0, 3))  # p, m, k, f


def prep_in_maps(inputs):
    h = np.asarray(inputs["hidden_states"], np.float32)
    pos = np.asarray(inputs["position_ids"])
    q_a_w = np.asarray(inputs["q_a_w"], np.float32)
    q_a_ln = np.asarray(inputs["q_a_ln"], np.float32)
    q_b_w = np.asarray(inputs["q_b_w"], np.float32)
    kv_a_w = np.asarray(inputs["kv_a_w"], np.float32)
    kv_a_ln = np.asarray(inputs["kv_a_ln"], np.float32)
    kv_b_w = np.asarray(inputs["kv_b_w"], np.float32)
    o_w = np.asarray(inputs["o_w"], np.float32)

    wqaP = _to_bf16_bits(_pack_stripes(q_a_w.T, NKT, NQL).reshape(128, NQL * NKT * 128))

    pe_rows_w = _interleave_rows(kv_a_w[KVL:])
    kv_a_w2 = np.concatenate([kv_a_w[:KVL], pe_rows_w, pe_rows_w], axis=0)
    wkvaP = _to_bf16_bits(_pack_stripes(kv_a_w2.T, NKT, 5).reshape(128, 5 * NKT * 128))

    scale = QHD ** -0.5
    rotP = _to_bf16_bits(_rot_matrix().T)

    per_core = []
    for c in range(8):
        b, g = divmod(c, 4)
        heads = range(HG * g, HG * g + HG)
        qsl = slice(g * QT, (g + 1) * QT)

        # q_b rows for this group (4 nope stripes + 2 pe pairs), ln+scale folded
        nope_rows = []
        pe_rows = []
        for hh in heads:
            rows = q_b_w[hh * QHD:(hh + 1) * QHD]
            nope_rows.append(rows[:NOPE])
            pe_rows.append(_interleave_rows(rows[NOPE:]))
        wqb_g = np.concatenate(nope_rows + pe_rows, axis=0)
        wqb_g = wqb_g * q_a_ln[None, :] * scale
        wqbP = _to_bf16_bits(_pack_stripes(wqb_g.T, NQL, 6).reshape(128, 6 * NQL * 128))

        kn_rows = []
        v_rows = []
        for hh in heads:
            rows = kv_b_w[hh * (NOPE + VD):(hh + 1) * (NOPE + VD)]
            kn_rows.append(rows[:NOPE])
            v_rows.append(rows[NOPE:])
        wkbn = (np.concatenate(kn_rows, axis=0) * kv_a_ln[None, :]).T
        wkbv = (np.concatenate(v_rows, axis=0) * kv_a_ln[None, :]).T
        kbnP = _to_bf16_bits(
            np.ascontiguousarray(wkbn.reshape(4, 128, GW).transpose(1, 0, 2)).reshape(128, 4 * GW))
        kbvP = _to_bf16_bits(
            np.ascontiguousarray(wkbv.reshape(4, 128, GW).transpose(1, 0, 2)).reshape(128, 4 * GW))

        wo = o_w[:, GW * g: GW * (g + 1)].T
        woP = _to_bf16_bits(_pack_stripes(wo, HG, NKT).reshape(128, NKT * HG * 128))

        # hidden transposed, quarter only: [128, 16, 256]
        hT = h[b].T[:, qsl]
        hTQ = hT.reshape(NKT, 128, QT).transpose(1, 0, 2)
        hTQ = _to_bf16_bits(np.ascontiguousarray(hTQ).reshape(128, NKT * QT))

        cos, sin = _rope_tables(np.asarray(pos[b]))
        cosP = _to_bf16_bits(np.concatenate([cos.T, cos.T], axis=0))  # [128, S]
        sinP = _to_bf16_bits(np.concatenate([sin.T, sin.T], axis=0))
        cosQ = np.ascontiguousarray(cosP[:, g * QT:(g + 1) * QT])
        sinQ = np.ascontiguousarray(sinP[:, g * QT:(g + 1) * QT])

        per_core.append({
            "hTQ": hTQ, "wqaP": wqaP, "wqbP": wqbP, "wkvaP": wkvaP,
            "kbnP": kbnP, "kbvP": kbvP, "woP": woP,
            "cosP": cosP, "sinP": sinP, "cosQ": cosQ, "sinQ": sinQ, "rotP": rotP,
        })
    return per_core


def combine_outputs(results):
    out = np.zeros((B, S, HID), np.float32)
    for c, r in enumerate(results):
        b = c // 4
        f32 = np.asarray(r["outP"]).astype(np.float32).reshape(128, 2, NKT, HJ)
        out[b] += f32.transpose(1, 3, 2, 0).reshape(S, HID)
    return out


def build_nc(debug=False, reps=1):
    nc = bacc.Bacc("TRN2", target_bir_lowering=False, debug=False, num_devices=8)
    dram = nc.declare_dram_parameter

    hTQ = dram("hTQ", [128, NKT * QT], BF16, isOutput=False)
    wqaP = dram("wqaP", [128, NQL * NKT * 128], BF16, isOutput=False)
    wqbP = dram("wqbP", [128, 6 * NQL * 128], BF16, isOutput=False)
    wkvaP = dram("wkvaP", [128, 5 * NKT * 128], BF16, isOutput=False)
    kbnP = dram("kbnP", [128, 4 * GW], BF16, isOutput=False)
    kbvP = dram("kbvP", [128, 4 * GW], BF16, isOutput=False)
    woP = dram("woP", [128, NKT * HG * 128], BF16, isOutput=False)
    cosP = dram("cosP", [128, S], BF16, isOutput=False)
    sinP = dram("sinP", [128, S], BF16, isOutput=False)
    cosQ = dram("cosQ", [128, QT], BF16, isOutput=False)
    sinQ = dram("sinQ", [128, QT], BF16, isOutput=False)
    rotP = dram("rotP", [128, 128], BF16, isOutput=False)
    outP = dram("outP", [128, 2 * NKT * HJ], BF16, isOutput=True)

    CHW = NST * QT  # 4352 cols contributed per rank
    cc_src = [nc.dram_tensor(f"cc_src{i}", [128, CHW], BF16) for i in range(2)]
    cc_dst = [nc.dram_tensor(f"cc_dst{i}", [4, 128, CHW], BF16) for i in range(2)]
    RG = [[0, 1, 2, 3], [4, 5, 6, 7]]

    AF = mybir.ActivationFunctionType
    MULT = mybir.AluOpType.mult
    ADD = mybir.AluOpType.add

    WQ = 3 * NKT * 128  # one wqa quarter (3 stripes)

    with tile.TileContext(nc) as tc:
        with (
            tc.tile_pool(name="consts", bufs=1) as consts,
            tc.tile_pool(name="sb", bufs=1) as sb,
            tc.tile_pool(name="ps", space="PSUM", bufs=1) as ps,
        ):
            # ---- resident weights (wqa streams per rep) ----
            wkva_sb = consts.tile([128, 5 * NKT * 128], BF16, name="wkva_sb")
            nc.sync.dma_start(wkva_sb[:], wkvaP[:, :])
            wqb_sb = consts.tile([128, 6 * NQL * 128], BF16, name="wqb_sb")
            nc.sync.dma_start(wqb_sb[:], wqbP[:, :])
            kbn_sb = consts.tile([128, 4 * GW], BF16, name="kbn_sb")
            nc.sync.dma_start(kbn_sb[:], kbnP[:, :])
            kbv_sb = consts.tile([128, 4 * GW], BF16, name="kbv_sb")
            nc.sync.dma_start(kbv_sb[:], kbvP[:, :])
            wo_sb = consts.tile([128, NKT * HG * 128], BF16, name="wo_sb")
            nc.sync.dma_start(wo_sb[:], woP[:, :])

            # ---- constants ----
            ones_f = consts.tile([128, 1], F32, name="ones_f")
            nc.vector.memset(ones_f[:], 1.0)
            ones_b = consts.tile([128, 1], BF16, name="ones_b")
            nc.vector.tensor_copy(ones_b[:], ones_f[:])
            eps_sb = consts.tile([128, 1], F32, name="eps_sb")
            nc.vector.memset(eps_sb[:], EPS)
            rot_sb = consts.tile([128, 128], BF16, name="rot_sb")
            nc.sync.dma_start(rot_sb[:], rotP[:, :])
            cos_sb = consts.tile([128, S], BF16, name="cos_sb")
            sin_sb = consts.tile([128, S], BF16, name="sin_sb")
            nc.sync.dma_start(cos_sb[:], cosP[:, :])
            nc.sync.dma_start(sin_sb[:], sinP[:, :])
            cosq_sb = consts.tile([128, QT], BF16, name="cosq_sb")
            sinq_sb = consts.tile([128, QT], BF16, name="sinq_sb")
            nc.sync.dma_start(cosq_sb[:], cosQ[:, :])
            nc.sync.dma_start(sinq_sb[:], sinQ[:, :])
            cmask = []
            for t in range(4):
                mt = consts.tile([128, HJ], BF16, name=f"cmask{t}")
                nc.gpsimd.memset(mt[:], 1.0)
                nc.gpsimd.affine_select(
                    out=mt[:], in_=mt[:],
                    compare_op=mybir.AluOpType.is_ge, fill=0.0,
                    base=-128 * t, pattern=[[1, HJ]], channel_multiplier=-1)
                cmask.append(mt)

            for sl in range(4):
                ei = sb.tile([128, HJ], BF16, name=f"et_init{sl}", tag="expT", bufs=4)
                nc.vector.memset(ei[:], 0.0)

            kT = [sb.tile([128, S], BF16, name=f"kT{hh}", tag="kT", bufs=4)
                  for hh in range(HG)]
            vsb = [sb.tile([128, GW], BF16, name=f"v{i}", tag="v", bufs=8)
                   for i in range(8)]

            def pre(rep):
                src = cc_src[rep % 2]
                dst = cc_dst[rep % 2]

                ht = sb.tile([128, NKT * QT], BF16, name=f"ht{rep}", tag="ht", bufs=2)
                nc.sync.dma_start(ht[:], hTQ[:, :])

                def htk(k):
                    return ht[:, k * QT:(k + 1) * QT]

                # ---- S1: q_lat^T quarter + sumsq; wqa streamed in quarters ----
                ql_t = []
                ps_msq = ps.tile([1, QT], F32, name=f"msq_q{rep}", tag="row", bufs=2)
                for mq in range(4):
                    wq = sb.tile([128, WQ], BF16, name=f"wqa{rep}_{mq}", tag="wqa", bufs=2)
                    nc.sync.dma_start(wq[:], wqaP[:, mq * WQ:(mq + 1) * WQ])
                    for mi in range(3):
                        m = mq * 3 + mi
                        pm = ps.tile([128, QT], F32, name=f"ps_qa{rep}_{m}", tag="mm", bufs=4)
                        for k in range(NKT):
                            nc.tensor.matmul(pm[:], wq[:, (mi * NKT + k) * 128:(mi * NKT + k + 1) * 128],
                                             htk(k), start=(k == 0), stop=(k == NKT - 1))
                        qt = sb.tile([128, QT], BF16, name=f"ql{rep}_{m}", tag="ql", bufs=NQL)
                        nc.vector.tensor_copy(qt[:], pm[:])
                        sqt = sb.tile([128, QT], BF16, name=f"sq_q{rep}_{m}", tag="sqt", bufs=2)
                        nc.scalar.activation(sqt[:], pm[:], AF.Square)
                        nc.tensor.matmul(ps_msq[:], ones_b[:], sqt[:],
                                         start=(m == 0), stop=(m == NQL - 1))
                        ql_t.append(qt)

                sr_q = sb.tile([1, QT], F32, name=f"sr_q{rep}", tag="srow", bufs=2)
                nc.scalar.activation(sr_q[:], ps_msq[:], AF.Sqrt, bias=eps_sb[0:1, :], scale=1.0 / QL)
                rr_q = sb.tile([1, QT], F32, name=f"rr_q{rep}", tag="srow", bufs=2)
                nc.vector.reciprocal(rr_q[:], sr_q[:])
                R_q = sb.tile([128, QT], F32, name=f"R_q{rep}", tag="bcast", bufs=2)
                nc.gpsimd.partition_broadcast(R_q[:], rr_q[:])
                for m in range(NQL):
                    nc.vector.tensor_tensor(out=ql_t[m][:], in0=ql_t[m][:], in1=R_q[:], op=MULT)

                # ---- S3: kv latent quarter + k_pe rope ----
                latn = []
                ps_msk = ps.tile([1, QT], F32, name=f"msq_kv{rep}", tag="row", bufs=2)
                for m in range(4):
                    pm = ps.tile([128, QT], F32, name=f"ps_kva{rep}_{m}", tag="mm", bufs=4)
                    for k in range(NKT):
                        nc.tensor.matmul(pm[:], wkva_sb[:, (m * NKT + k) * 128:(m * NKT + k + 1) * 128],
                                         htk(k), start=(k == 0), stop=(k == NKT - 1))
                    lt = sb.tile([128, QT], BF16, name=f"latn{rep}_{m}", tag="latn", bufs=4)
                    nc.vector.tensor_copy(lt[:], pm[:])
                    latn.append(lt)
                    sqt = sb.tile([128, QT], BF16, name=f"sq_kv{rep}_{m}", tag="sqt", bufs=2)
                    nc.scalar.activation(sqt[:], pm[:], AF.Square)
                    nc.tensor.matmul(ps_msk[:], ones_b[:], sqt[:],
                                     start=(m == 0), stop=(m == 3))
                sr_k = sb.tile([1, QT], F32, name=f"sr_k{rep}", tag="srow", bufs=2)
                nc.scalar.activation(sr_k[:], ps_msk[:], AF.Sqrt, bias=eps_sb[0:1, :], scale=1.0 / KVL)
                rr_k = sb.tile([1, QT], F32, name=f"rr_k{rep}", tag="srow", bufs=2)
                nc.vector.reciprocal(rr_k[:], sr_k[:])
                R_kv = sb.tile([128, QT], F32, name=f"R_kv{rep}", tag="bcast", bufs=2)
                nc.gpsimd.partition_broadcast(R_kv[:], rr_k[:])
                for m in range(4):
                    nc.vector.tensor_tensor(out=latn[m][:], in0=latn[m][:], in1=R_kv[:], op=MULT)

                pm = ps.tile([128, QT], F32, name=f"ps_pe{rep}", tag="mm", bufs=4)
                for k in range(NKT):
                    nc.tensor.matmul(pm[:], wkva_sb[:, (4 * NKT + k) * 128:(4 * NKT + k + 1) * 128],
                                     htk(k), start=(k == 0), stop=(k == NKT - 1))
                xpe = sb.tile([128, QT], BF16, name=f"xpe{rep}", tag="tmp", bufs=4)
                nc.vector.tensor_copy(xpe[:], pm[:])
                pr = ps.tile([128, QT], F32, name=f"ps_rot{rep}", tag="mm", bufs=4)
                nc.tensor.matmul(pr[:], rot_sb[:], xpe[:], start=True, stop=True)
                t1 = sb.tile([128, QT], BF16, name=f"t1k{rep}", tag="tmp", bufs=4)
                nc.vector.tensor_tensor(out=t1[:], in0=xpe[:], in1=cosq_sb[:], op=MULT)
                t2 = sb.tile([128, QT], BF16, name=f"t2k{rep}", tag="tmp", bufs=4)
                nc.vector.tensor_tensor(out=t2[:], in0=pr[:], in1=sinq_sb[:], op=MULT)
                kpe_q = sb.tile([128, QT], BF16, name=f"kpe_q{rep}", tag="kpe_q", bufs=2)
                nc.vector.tensor_tensor(out=kpe_q[:], in0=t1[:], in1=t2[:], op=ADD)

                # ---- AllGather the 17 stripes ----
                for m in range(NQL):
                    nc.sync.dma_start(src[:, m * QT:(m + 1) * QT], ql_t[m][:])
                for m in range(4):
                    nc.sync.dma_start(src[:, (NQL + m) * QT:(NQL + m + 1) * QT], latn[m][:])
                nc.sync.dma_start(src[:, 16 * QT:17 * QT], kpe_q[:])
                nc.gpsimd.collective_compute(
                    "AllGather",
                    mybir.AluOpType.bypass,
                    ins=[src[:, :]],
                    outs=[dst[:, :, :]],
                    replica_groups=RG,
                )

            def post(rep):
                dst = cc_dst[rep % 2]
                # gathered stripes, loaded just-in-time from DRAM:
                # glat (latn, 4 stripes) first for S4/S5, kpeT its own small tile,
                # gql (12 ql stripes) for S2.
                glat = sb.tile([128, 4 * S], BF16, name=f"glat{rep}", tag="glat", bufs=1)
                for r in range(4):
                    nc.sync.dma_start(
                        glat[:].rearrange("p (s t) -> p s t", s=4)[:, :, r * QT:(r + 1) * QT],
                        dst[r, :, NQL * QT:16 * QT].rearrange("p (s t) -> p s t", s=4))
                kpeT = sb.tile([128, S], BF16, name=f"kpeT{rep}", tag="kpeT", bufs=2)
                for r in range(4):
                    nc.sync.dma_start(kpeT[:, r * QT:(r + 1) * QT], dst[r, :, 16 * QT:17 * QT])
                gql = sb.tile([128, NQL * S], BF16, name=f"gql{rep}", tag="gql", bufs=1)
                for r in range(4):
                    nc.sync.dma_start(
                        gql[:].rearrange("p (s t) -> p s t", s=NQL)[:, :, r * QT:(r + 1) * QT],
                        dst[r, :, 0:NQL * QT].rearrange("p (s t) -> p s t", s=NQL))

                def gs(s):
                    # stripe view: 0..11 ql, 12..15 latn
                    if s < NQL:
                        return gql[:, s * S:(s + 1) * S]
                    return glat[:, (s - NQL) * S:(s - NQL + 1) * S]

                for j in range(2):
                    jsl = slice(j * HJ, (j + 1) * HJ)

                    # ---- S4: k_nope^T per head ----
                    for hh in range(HG):
                        pm = ps.tile([128, HJ], F32, name=f"ps_kn{rep}{j}_{hh}", tag="mm", bufs=4)
                        for k4 in range(4):
                            nc.tensor.matmul(pm[:], kbn_sb[:, k4 * GW + hh * 128:k4 * GW + (hh + 1) * 128],
                                             gs(NQL + k4)[:, jsl], start=(k4 == 0), stop=(k4 == 3))
                        nc.scalar.copy(kT[hh][:, jsl], pm[:])

                    # ---- S5: v (natural layout) ----
                    for tt in range(4):
                        i = 4 * j + tt
                        csl = slice(j * HJ + tt * 128, j * HJ + (tt + 1) * 128)
                        pm = ps.tile([128, GW], F32, name=f"ps_v{rep}_{i}", tag="mm", bufs=4)
                        for k4 in range(4):
                            nc.tensor.matmul(pm[:], gs(NQL + k4)[:, csl], kbv_sb[:, k4 * GW:(k4 + 1) * GW],
                                             start=(k4 == 0), stop=(k4 == 3))
                        nc.scalar.copy(vsb[i][:], pm[:])

                    # ---- S2: q^T stripes for own heads (no R_q: ql pre-normalized) ----
                    qT = []
                    for m in range(6):
                        pm = ps.tile([128, HJ], F32, name=f"ps_qb{rep}{j}_{m}", tag="mm", bufs=4)
                        for k in range(NQL):
                            nc.tensor.matmul(pm[:], wqb_sb[:, (m * NQL + k) * 128:(m * NQL + k + 1) * 128],
                                             gs(k)[:, jsl], start=(k == 0), stop=(k == NQL - 1))
                        qt = sb.tile([128, HJ], BF16, name=f"qT{rep}{j}_{m}", tag="qT", bufs=6)
                        if m < 4:
                            nc.vector.tensor_copy(qt[:], pm[:])
                        else:
                            xq = sb.tile([128, HJ], BF16, name=f"xq{rep}{j}_{m}", tag="tmp", bufs=4)
                            nc.vector.tensor_copy(xq[:], pm[:])
                            prq = ps.tile([128, HJ], F32, name=f"ps_rotq{rep}{j}_{m}", tag="mm", bufs=4)
                            nc.tensor.matmul(prq[:], rot_sb[:], xq[:], start=True, stop=True)
                            t1q = sb.tile([128, HJ], BF16, name=f"t1q{rep}{j}_{m}", tag="tmp", bufs=4)
                            nc.vector.tensor_tensor(out=t1q[:], in0=xq[:], in1=cos_sb[:, jsl], op=MULT)
                            t2q = sb.tile([128, HJ], BF16, name=f"t2q{rep}{j}_{m}", tag="tmp", bufs=4)
                            nc.vector.tensor_tensor(out=t2q[:], in0=prq[:], in1=sin_sb[:, jsl], op=MULT)
                            nc.vector.tensor_tensor(out=qt[:], in0=t1q[:], in1=t2q[:], op=ADD)
                        qT.append(qt)

                    # ---- attention per head ----
                    attn = []
                    for hh in range(HG):
                        qpe = qT[4 + hh // 2][(hh % 2) * 64:(hh % 2) * 64 + 64, :]
                        pe0 = (hh % 2) * 64
                        po = ps.tile([128, HJ], F32, name=f"ps_o{rep}{j}_{hh}", tag="acc", bufs=2)
                        psum = ps.tile([1, HJ], F32, name=f"ps_sum{rep}{j}_{hh}", tag="row", bufs=2)
                        irange = list(range(4 * (j + 1)))
                        last = irange[-1]
                        for i in irange:
                            t = i - 4 * j
                            q0 = t * 128 if t > 0 else 0
                            qs = slice(q0, HJ)
                            pss = ps.tile([128, HJ], F32, name=f"ps_s{rep}{j}_{hh}_{i}", tag="mm", bufs=4)
                            nc.tensor.matmul(pss[:, qs], kT[hh][:, i * 128:(i + 1) * 128], qT[hh][:, qs],
                                             start=True, stop=False)
                            nc.tensor.matmul(pss[:, qs], kpeT[pe0:pe0 + 64, i * 128:(i + 1) * 128],
                                             qpe[:, qs], start=False, stop=True)
                            et = sb.tile([128, HJ], BF16, name=f"e{rep}{j}_{hh}_{i}", tag="expT", bufs=4)
                            nc.scalar.activation(et[:, qs], pss[:, qs], AF.Exp)
                            if i * 128 + 127 > j * HJ:
                                nc.vector.tensor_tensor(out=et[:], in0=et[:], in1=cmask[i - 4 * j][:], op=MULT)
                            nc.tensor.matmul(psum[:], ones_b[:], et[:],
                                             start=(i == 0), stop=(i == last))
                            nc.tensor.matmul(po[:], vsb[i][:, hh * 128:(hh + 1) * 128], et[:],
                                             start=(i == 0), stop=(i == last))
                        rs = sb.tile([1, HJ], F32, name=f"rs{rep}{j}_{hh}", tag="srow", bufs=2)
                        nc.vector.reciprocal(rs[:], psum[:])
                        Rs = sb.tile([128, HJ], F32, name=f"Rs{rep}{j}_{hh}", tag="bcast", bufs=2)
                        nc.gpsimd.partition_broadcast(Rs[:], rs[:])
                        at = sb.tile([128, HJ], BF16, name=f"attn{rep}{j}_{hh}", tag="attn", bufs=4)
                        nc.vector.tensor_tensor(out=at[:], in0=po[:], in1=Rs[:], op=MULT)
                        attn.append(at)

                    # ---- o_proj ----
                    for oc4 in range(4):
                        ot = sb.tile([128, 2 * HJ], BF16, name=f"ot{rep}{j}_{oc4}a", tag="osb", bufs=4)
                        ot2 = sb.tile([128, 2 * HJ], BF16, name=f"ot{rep}{j}_{oc4}b", tag="osb", bufs=4)
                        for q4 in range(4):
                            oc = oc4 * 4 + q4
                            pm = ps.tile([128, HJ], F32, name=f"ps_out{rep}{j}_{oc}", tag="mm", bufs=4)
                            for hh in range(HG):
                                nc.tensor.matmul(pm[:], wo_sb[:, oc * GW + hh * 128:oc * GW + (hh + 1) * 128],
                                                 attn[hh][:], start=(hh == 0), stop=(hh == HG - 1))
                            dstt = ot if q4 < 2 else ot2
                            nc.vector.tensor_copy(dstt[:, (q4 % 2) * HJ:(q4 % 2 + 1) * HJ], pm[:])
                        nc.sync.dma_start(
                            outP[:, (j * NKT + oc4 * 4) * HJ:(j * NKT + oc4 * 4 + 2) * HJ],
                            ot[:])
                        nc.sync.dma_start(
                            outP[:, (j * NKT + oc4 * 4 + 2) * HJ:(j * NKT + oc4 * 4 + 4) * HJ],
                            ot2[:])

            pre(0)
            for rep in range(reps):
                if rep + 1 < reps:
                    pre(rep + 1)
                post(rep)

    nc.compile()
    return nc


_NC = None


def _get_nc():
    global _NC
    if _NC is None:
        _NC = build_nc()
    return _NC


def run(inputs, trace=False):
    in_maps = prep_in_maps(inputs)
    nc = _get_nc()
    res = run_bass_kernel_spmd(nc, in_maps, core_ids=list(range(8)), trace=trace)
    out = combine_outputs(res.results)
    return out, res


def kernel(**inputs):
    out, _ = run(inputs)
    return out.astype(np.float32)
